# revision 22
# baseline (speedup 1.0000x reference)
"""Trainium2 Bass kernel for nn_CrossAttentionLayer (2-stream cross-attention + LN).

Sharding: 8 cores = (stream s in {0,1}) x (batch b in {0,1}) x (query chunk c in {0,1}).
Each core handles 1024 query tokens of one (stream, batch): it projects Q for its
tokens, K/V for the *other* stream's full 2048 tokens (data-parallel duplication of
KV-proj across the 2 chunk cores), runs 16-head cross attention, out-projection,
residual and LayerNorm, and returns its [1024, 1024] slice.

Perf scheme (v2):
- All projections (Q/K/V/out) and the AV matmul run in fp8e4 with DoubleRow
  perf mode (2 contraction subtiles per instruction, 0.5 cycles/row = 2x fp16
  matmul throughput). Scores stay at 1.0 cycles/row (contraction=64 head dim),
  operands fp8.
- Weights are pre-scaled on the host so fp8e4 (max 240, min normal 2^-6) sees
  ~unit-scale values: Wq,Wk x64, Wv,Wout x32. Scale bookkeeping:
  Q' K' = 4096 QK -> folded into the exp() scale argument; V' = 32V and
  Wout' = 32 Wout -> attn-out psum = 1024 attn@Wout, matched by x_own and bout
  pre-scaled x1024 on the host. The final LayerNorm is scale-invariant, so the
  output is exact.
- exp() on the ACT engine writes fp8 directly (the ACT engine is the critical
  path: 33.5M softmax exps/core ~ 220us minimum).
- Softmax normalization: reciprocal_approx_fast (5x faster than
  nc.vector.reciprocal) + gpsimd partition_broadcast (Pool engine) instead of
  a DRAM bounce.
- Single interleaved region: K(f)/Q(f) projections feed scores for head pair
  g=f immediately, so the ACT engine starts exp'ing ~10us in and stays busy;
  V projection and AV/out-proj ride in PE slack under the exp stream.
"""

import os
import sys

import numpy as np

for _p in ("/opt/trn_rl_repo", "/root/.axon_site/_ro/trn_rl_repo"):
    if os.path.isdir(_p) and _p not in sys.path:
        sys.path.insert(0, _p)

import ml_dtypes

import concourse.bass as bass
import concourse.mybir as mybir
import concourse.tile as tile
from concourse.bass_utils import run_bass_kernel_spmd

F32 = mybir.dt.float32
F16 = mybir.dt.float16
F8 = mybir.dt.float8e4
NPF8 = ml_dtypes.float8_e4m3
ADD = mybir.AluOpType.add
MULT = mybir.AluOpType.mult
EXP = mybir.ActivationFunctionType.Exp
SQRT = mybir.ActivationFunctionType.Sqrt
DR = mybir.MatmulPerfMode.DoubleRow

DIM = 1024
N_TOK = 2048
HEADS = 16
HD = DIM // HEADS        # 64
NQ = 1024                # query tokens per core
S = 2048                 # kv sequence length
P = 128
DT = DIM // P            # 8 contraction tiles
DTP = DT // 2            # 4 contraction tile-pairs (DoubleRow)
FT = DIM // P            # 8 feature tiles
KT = S // P              # 16 key tiles
KT2 = KT // 2            # 8 key tile-pairs
NB = 512                 # matmul free-dim / psum bank width (fp32)
QC = NQ // NB            # 2 query chunks
KC = S // NB             # 4 key chunks
TT = NQ // P             # 8 token tiles per core
SCALE = HD ** -0.5
EPS = 1e-5

# host-side fp8 pre-scales (powers of two; see module docstring)
SC_QK = 64.0             # Wq, Wk, bq, bk
SC_V = 32.0              # Wv, bv
SC_WO = 32.0             # Wout
SC_X = SC_V * SC_WO      # x_own, bout (1024)
EXP_SCALE = SCALE / (SC_QK * SC_QK)

_wsplit_ctr = [0]


def _ensure_ntff_hook():
    """Register the axon NTFF profiling hook if the image lacks
    antenv.axon_hooks (mirrors trn_boot._ntff_profile_via_ctypes)."""
    try:
        from antenv.axon_hooks import get_axon_ntff_profile_hook  # noqa: F401
        return
    except ImportError:
        pass
    import contextlib
    import ctypes
    import types

    try:
        import antenv
    except ImportError:
        return
    mod = types.ModuleType("antenv.axon_hooks")
    _h = [None]
    mod.set_axon_ntff_profile_hook = lambda h: _h.__setitem__(0, h)
    mod.get_axon_ntff_profile_hook = lambda: _h[0]
    sys.modules["antenv.axon_hooks"] = mod
    antenv.axon_hooks = mod

    so_path = "/opt/axon/libaxon_pjrt.so"
    if not os.path.exists(so_path):
        return
    try:
        lib = ctypes.CDLL(so_path)
    except OSError:
        return
    if not hasattr(lib, "axon_start_nrt_profile"):
        return
    lib.axon_start_nrt_profile.argtypes = [
        ctypes.POINTER(ctypes.c_int64),
        ctypes.c_size_t,
    ]
    lib.axon_start_nrt_profile.restype = ctypes.c_int64
    lib.axon_stop_nrt_profile.argtypes = [ctypes.c_char_p]
    lib.axon_stop_nrt_profile.restype = ctypes.c_int64

    @contextlib.contextmanager
    def _hook(output_dir, device_ids):
        import jax

        jax.devices()
        if device_ids:
            ids = (ctypes.c_int64 * len(device_ids))(*device_ids)
            rc = lib.axon_start_nrt_profile(ids, len(device_ids))
        else:
            rc = lib.axon_start_nrt_profile(None, 0)
        if rc != 0:
            raise RuntimeError(f"axon_start_nrt_profile rc={rc}")
        try:
            yield
        finally:
            n = lib.axon_stop_nrt_profile(str(output_dir).encode())
            if n <= 0:
                print(f"profile: rc={n}, no ntff written to {output_dir}")

    mod.set_axon_ntff_profile_hook(_hook)


def _patch_upload_artifacts():
    """Artifact upload needs bucket access this container may not have;
    neuter it (only reachable on trace paths)."""
    from concourse import bass_utils as bu

    bu.upload_artifacts = lambda tmpdir: str(tmpdir)


def _split_sync_waits(nc):
    """This container's walrus build rejects >1 sync-wait per instruction.
    Hoist extra waits onto same-engine NOPs placed just before the instruction
    (engines execute their stream in order, so semantics are preserved)."""
    for f in nc.m.functions:
        for bb in f.blocks:
            insts = bb.instructions
            out = []
            changed = False
            for inst in insts:
                si = inst.sync_info
                if si is not None and si.on_wait and len(si.on_wait) > 1:
                    waits = list(si.on_wait)
                    for w in waits[:-1]:
                        _wsplit_ctr[0] += 1
                        out.append(
                            mybir.InstNoOp(
                                name=f"I-wsplit-{_wsplit_ctr[0]}",
                                engine=inst.engine,
                                ins=[],
                                outs=[],
                                sync_info=mybir.SyncInfo(on_wait=[w], on_update=[]),
                            )
                        )
                    si.on_wait = waits[-1:]
                    changed = True
                out.append(inst)
            if changed:
                insts[:] = out
    return nc


def _build_bass():
    nc = bass.Bass()
    x_own = nc.declare_dram_parameter("x_own", [NQ, DIM], F16, isOutput=False)
    xT8 = nc.declare_dram_parameter("xT8", [DIM, NQ], F8, isOutput=False)
    xoT8 = nc.declare_dram_parameter("xoT8", [DIM, S], F8, isOutput=False)
    wqkv8 = nc.declare_dram_parameter("wqkv8", [DIM, 3 * DIM], F8, isOutput=False)
    wout8i = nc.declare_dram_parameter("wout8i", [DIM, DIM], F8, isOutput=False)
    bqkv = nc.declare_dram_parameter("bqkv", [3 * DIM], F32, isOutput=False)
    bout = nc.declare_dram_parameter("bout", [1, DIM], F32, isOutput=False)
    gamma = nc.declare_dram_parameter("gamma", [1, DIM], F16, isOutput=False)
    beta = nc.declare_dram_parameter("beta", [1, DIM], F16, isOutput=False)
    y_ext = nc.declare_dram_parameter("y", [NQ, DIM], F16, isOutput=True)

    with tile.TileContext(nc, pool_alloc_mode="queue") as tc:
        from contextlib import ExitStack

        with ExitStack() as ctx:
            const = ctx.enter_context(tc.tile_pool(name="const", bufs=1))
            persist = ctx.enter_context(tc.tile_pool(name="persist", bufs=1))
            dram = ctx.enter_context(tc.tile_pool(name="dram", bufs=1, space="DRAM"))
            dramn = ctx.enter_context(tc.tile_pool(name="dramn", bufs=3, space="DRAM"))
            # PSUM during attention: psP 2 + psS 2*2 + psAt 2 = 8 banks
            psP = ctx.enter_context(tc.tile_pool(name="psP", bufs=2, space="PSUM"))
            pTp = ctx.enter_context(tc.tile_pool(name="pT", bufs=32))
            rrp = ctx.enter_context(tc.tile_pool(name="rr", bufs=4))
            rdp = ctx.enter_context(tc.tile_pool(name="rd", bufs=4))
            asg = ctx.enter_context(tc.tile_pool(name="asg", bufs=5))
            stE = ctx.enter_context(tc.tile_pool(name="stE", bufs=3))
            xrp = ctx.enter_context(tc.tile_pool(name="xrp", bufs=8))
            aTE = ctx.enter_context(tc.tile_pool(name="aTE", bufs=8))

            # ---- constants (broadcast along partitions via DMA) ----
            bq_cols = const.tile([P, 3 * DT], F32)  # bqkv as feat-major columns
            nc.sync.dma_start(out=bq_cols[:], in_=bqkv[:].rearrange("(t p) -> p t", p=P))
            bv_rep = const.tile([P, DIM], F32)
            nc.sync.dma_start(
                out=bv_rep[:],
                in_=bass.AP(tensor=bqkv[:].tensor, offset=2 * DIM, ap=[[0, P], [1, DIM]]),
            )
            gamma_rep = const.tile([P, DIM], F16)
            nc.sync.dma_start(out=gamma_rep[:], in_=gamma[:].to_broadcast([P, DIM]))
            beta_rep = const.tile([P, DIM], F16)
            nc.sync.dma_start(out=beta_rep[:], in_=beta[:].to_broadcast([P, DIM]))
            eps_t = const.tile([P, 1], F32)
            nc.vector.memset(eps_t[:], EPS)

            # ---- persistent operands ----
            # K/Q stored fp8, feature-major [feat 128, tokens]
            kTs = [persist.tile([P, S], F8, name=f"kT{f}") for f in range(FT)]
            qTs = [persist.tile([P, NQ], F8, name=f"qT{f}") for f in range(FT)]
            # V in fp8, DoubleRow pair layout: [key 128, ktile j, head, hd+1]
            vSs = [persist.tile([P, 2, HEADS, HD + 1], F8, name=f"vS{k2}")
                   for k2 in range(KT2)]
            # fp8 inputs (DoubleRow pair layout [128, 2, n])
            wk8a = persist.tile([P, DT, DIM], F8, name="wk8a")
            wq8a = persist.tile([P, DT, DIM], F8, name="wq8a")
            wv8a = persist.tile([P, DT, DIM], F8, name="wv8a")
            wo8a = persist.tile([P, DT, DIM], F8, name="wo8a")
            xo8a = persist.tile([P, DT, S], F8, name="xo8a")
            x8a = persist.tile([P, DT, NQ], F8, name="x8a")
            wk8 = [wk8a[:, 2 * dp:2 * dp + 2, :] for dp in range(DTP)]
            wq8 = [wq8a[:, 2 * dp:2 * dp + 2, :] for dp in range(DTP)]
            wv8 = [wv8a[:, 2 * dp:2 * dp + 2, :] for dp in range(DTP)]
            wo8 = [wo8a[:, 2 * dp:2 * dp + 2, :] for dp in range(DTP)]
            xo8 = [xo8a[:, 2 * dp:2 * dp + 2, :] for dp in range(DTP)]
            x8 = [x8a[:, 2 * dp:2 * dp + 2, :] for dp in range(DTP)]

            # ---- input DMA loads: one DMA per tensor, spread across issue
            # queues; K-path (sync) first so scores start earliest ----
            nc.sync.dma_start(
                out=xo8a[:], in_=xoT8[:, :].rearrange("(r p) n -> p r n", p=P))
            nc.sync.dma_start(
                out=wk8a[:],
                in_=wqkv8[:, DIM:2 * DIM].rearrange("(r p) n -> p r n", p=P))
            nc.scalar.dma_start(
                out=wq8a[:],
                in_=wqkv8[:, 0:DIM].rearrange("(r p) n -> p r n", p=P))
            nc.scalar.dma_start(
                out=x8a[:], in_=xT8[:, :].rearrange("(r p) n -> p r n", p=P))
            nc.gpsimd.dma_start(
                out=wv8a[:],
                in_=wqkv8[:, 2 * DIM:3 * DIM].rearrange("(r p) n -> p r n", p=P))
            nc.gpsimd.dma_start(
                out=wo8a[:], in_=wout8i[:, :].rearrange("(r p) n -> p r n", p=P))
            # residual rows: independent of everything, preload now
            xrp_t = []
            for t in range(TT):
                xr = xrp.tile([P, DIM], F16, tag="xr", name=f"xr{t}")
                nc.gpsimd.dma_start(out=xr[:], in_=x_own[t * P:(t + 1) * P, :])
                xrp_t.append(xr)

            attn_d = dram.tile([DIM, NQ], F8, name="attnd")

            # ones rows for the softmax denominator (independent bytes of vSs)
            for k2 in range(KT2):
                nc.vector.memset(vSs[k2][:, :, :, HD:HD + 1], 1.0)

            # ---- filler units: single-shot closures emitting ~4 PE matmuls
            # each; woven between exp-paced score rounds to keep the PE dense.
            from collections import deque
            fillers = deque()

            def drain(n):
                for _ in range(n):
                    if fillers:
                        fillers.popleft()()

            def proj_k_unit(f, kc):
                def emit():
                    ps = psP.tile([P, NB], F32, tag="ps", name="ps")
                    for dp in range(DTP):
                        nc.tensor.matmul(
                            ps[:],
                            lhsT=wk8[dp][:, :, f * P:(f + 1) * P],
                            rhs=xo8[dp][:, :, kc * NB:(kc + 1) * NB],
                            start=(dp == 0),
                            stop=(dp == DTP - 1),
                            perf_mode=DR,
                        )
                    nc.vector.tensor_scalar(
                        out=kTs[f][:, kc * NB:(kc + 1) * NB],
                        in0=ps[:],
                        scalar1=bq_cols[:, DT + f:DT + f + 1],
                        scalar2=None,
                        op0=ADD,
                    )
                return emit

            def proj_q_unit(f, q):
                def emit():
                    ps = psP.tile([P, NB], F32, tag="ps", name="ps")
                    for dp in range(DTP):
                        nc.tensor.matmul(
                            ps[:],
                            lhsT=wq8[dp][:, :, f * P:(f + 1) * P],
                            rhs=x8[dp][:, :, q * NB:(q + 1) * NB],
                            start=(dp == 0),
                            stop=(dp == DTP - 1),
                            perf_mode=DR,
                        )
                    nc.vector.tensor_scalar(
                        out=qTs[f][:, q * NB:(q + 1) * NB],
                        in0=ps[:],
                        scalar1=bq_cols[:, f:f + 1],
                        scalar2=None,
                        op0=ADD,
                    )
                return emit

            def proj_v_unit(k2, j, half):
                kt = 2 * k2 + j

                def emit():
                    ps = psP.tile([P, NB], F32, tag="ps", name="ps")
                    for dp in range(DTP):
                        nc.tensor.matmul(
                            ps[:],
                            lhsT=xo8[dp][:, :, kt * P:(kt + 1) * P],
                            rhs=wv8[dp][:, :, half * NB:(half + 1) * NB],
                            start=(dp == 0),
                            stop=(dp == DTP - 1),
                            perf_mode=DR,
                        )
                    nc.vector.tensor_add(
                        vSs[k2][:, j, half * 8:(half + 1) * 8, 0:HD],
                        ps[:].rearrange("p (h d) -> p h d", d=HD),
                        bv_rep[:, half * NB:(half + 1) * NB].rearrange(
                            "p (h d) -> p h d", d=HD
                        ),
                    )
                return emit

            def av_unit(g, q, hi, pts, sink):
                def emit():
                    ps_at = psAt.tile([HD + 1, NB], F32, tag="psa",
                                      name=f"psa{g}_{q}_{hi}")
                    for k2 in range(KT2):
                        nc.tensor.matmul(
                            ps_at[:],
                            lhsT=vSs[k2][:, :, 2 * g + hi, :],
                            rhs=pts[k2][hi][:],
                            start=(k2 == 0),
                            stop=(k2 == KT2 - 1),
                            perf_mode=DR,
                        )
                    a32 = asg.tile([HD + 1, NB], F32, tag="a32", name="a32")
                    nc.vector.tensor_copy(a32[:], ps_at[:])
                    sink["a32"][(q, hi)] = a32
                    nc.gpsimd.dma_start(
                        out=sink["den_d"][2 * q + hi:2 * q + hi + 1, :],
                        in_=a32[HD:HD + 1, :],
                    )
                return emit

            def norm_unit(g, sink):
                """Batched reciprocal of this head pair's 4 denominators via a
                DRAM bounce reshaped to [128, 16], then normalize + store."""
                def emit():
                    rd4 = rdp.tile([P, 16], F32, tag="rd4", name="rd4")
                    dflat = sink["den_d"][:]
                    nc.gpsimd.dma_start(
                        out=rd4[:],
                        in_=bass.AP(tensor=dflat.tensor, offset=dflat.offset,
                                    ap=[[16, P], [1, 16]]),
                    )
                    nc.vector.reciprocal(rd4[:], rd4[:])
                    rec_d = dramn.tile([4, NB], F32, tag="recd", name="recd")
                    rflat = rec_d[:]
                    nc.gpsimd.dma_start(
                        out=bass.AP(tensor=rflat.tensor, offset=rflat.offset,
                                    ap=[[16, P], [1, 16]]),
                        in_=rd4[:],
                    )
                    for q in range(QC):
                        qsl = slice(q * NB, (q + 1) * NB)
                        for hi in range(2):
                            h = 2 * g + hi
                            rrep = rrp.tile([HD, NB], F32, tag="rrep", name="rrep")
                            nc.gpsimd.dma_start(
                                out=rrep[:],
                                in_=rec_d[2 * q + hi:2 * q + hi + 1, :]
                                .to_broadcast([HD, NB]),
                            )
                            a8 = asg.tile([HD, NB], F8, tag="a8", name="a8")
                            nc.vector.tensor_mul(
                                a8[:], sink["a32"][(q, hi)][0:HD, :], rrep[:]
                            )
                            nc.gpsimd.dma_start(
                                out=attn_d[h * HD:(h + 1) * HD, qsl], in_=a8[:]
                            )
                return emit

            def queue_av(g, pts_q):
                den_d = dramn.tile([4, NB], F32, tag="dend", name="dend")
                sink = {"a32": {}, "den_d": den_d}
                for q in range(QC):
                    for hi in range(2):
                        fillers.append(av_unit(g, q, hi, pts_q[q], sink))
                return sink

            with tc.tile_pool(name="psS", bufs=2, space="PSUM") as psS, \
                 tc.tile_pool(name="psAt", bufs=2, space="PSUM") as psAt:

                def scores(g, q, pts, per_round=1):
                    """QK^T + exp for head pair g, query chunk q; filler
                    units woven between kt2 rounds."""
                    f = g
                    qsl = slice(q * NB, (q + 1) * NB)
                    sched = (per_round if isinstance(per_round, list)
                             else [per_round] * KT2)
                    for k2 in range(KT2):
                        ps_s = [
                            psS.tile([P, 2, NB], F32, tag="pss",
                                     name=f"pss{g}_{q}_{k2}_{i}")
                            for i in range(2)
                        ]
                        for j in range(2):
                            kt = 2 * k2 + j
                            for hi in range(2):
                                po = hi * HD
                                nc.tensor.matmul(
                                    ps_s[hi][:, j, :],
                                    lhsT=kTs[f][po:po + HD, kt * P:(kt + 1) * P],
                                    rhs=qTs[f][po:po + HD, qsl],
                                    start=True,
                                    stop=True,
                                )
                        drain(sched[k2])
                        pp = []
                        for hi in range(2):
                            pt = pTp.tile([P, 2, NB], F8, tag="pT", name="pt")
                            nc.scalar.activation(pt[:], ps_s[hi][:], EXP,
                                                 scale=EXP_SCALE)
                            pp.append(pt)
                        pts.append(pp)

                # ---- weave driver ----
                # Ordering invariants (PE queue is in-order, so a stalled
                # instruction blocks everything behind it):
                #  - K(g)/Q(g) units fully emitted before scores(g) starts.
                #  - AV(g-1) units emitted in the FIRST rounds of scores(g):
                #    exp(g) tiles WAR-wait on the pt ring buffers that
                #    AV(g-1) reads (ring size = one head pair's tiles).
                #  - All V units emitted before AV(0) (drain 2/round in S(0)).
                pts_all = {}
                for kc in range(KC):
                    proj_k_unit(0, kc)()
                for q in range(QC):
                    proj_q_unit(0, q)()
                for k2 in range(KT2):
                    for j in range(2):
                        for half in range(2):
                            fillers.append(proj_v_unit(k2, j, half))
                fillers.append(proj_k_unit(1, 0))
                fillers.append(proj_k_unit(1, 1))
                fillers.append(proj_k_unit(1, 2))
                fillers.append(proj_k_unit(1, 3))
                fillers.append(proj_q_unit(1, 0))
                fillers.append(proj_q_unit(1, 1))
                pts_all[(0, 0)] = []
                scores(0, 0, pts_all[(0, 0)], per_round=[0, 0, 1, 2, 3, 3, 3, 3])
                pts_all[(0, 1)] = []
                scores(0, 1, pts_all[(0, 1)], per_round=3)
                for g in range(1, HEADS // 2):
                    drain(len(fillers))  # flush: K(g)/Q(g) done before S(g)
                    sink = queue_av(g - 1, [pts_all[(g - 1, q)] for q in range(QC)])
                    if g + 1 < HEADS // 2:
                        for kc in range(KC):
                            fillers.append(proj_k_unit(g + 1, kc))
                        for q in range(QC):
                            fillers.append(proj_q_unit(g + 1, q))
                    fillers.append(norm_unit(g - 1, sink))
                    for q in range(QC):
                        pts_all[(g, q)] = []
                        scores(g, q, pts_all[(g, q)])
                    for q in range(QC):
                        pts_all.pop((g - 1, q))
                sink = queue_av(HEADS // 2 - 1,
                                [pts_all[(HEADS // 2 - 1, q)] for q in range(QC)])
                fillers.append(norm_unit(HEADS // 2 - 1, sink))
                drain(len(fillers))

            # ======== out proj + residual + LN ========
            psE = ctx.enter_context(tc.tile_pool(name="psE", bufs=3, space="PSUM"))
            aT6s = []
            for t in range(TT):
                aT6 = aTE.tile([P, FT, P], F8, tag="aT", name=f"aT6_{t}")
                nc.sync.dma_start(
                    out=aT6[:],
                    in_=attn_d[:].rearrange("(f p) t -> p f t", p=P)
                    [:, :, t * P:(t + 1) * P],
                )
                aT6s.append(aT6)
            for t in range(TT):
                tsl = slice(t * P, (t + 1) * P)
                x32 = xrp_t[t]
                y_sb = stE.tile([P, DIM], F16, tag="ysb")
                aT6 = aT6s[t]
                for half in range(2):
                    ps = psE.tile([P, NB], F32, tag="ps")
                    for dp in range(DTP):
                        nc.tensor.matmul(
                            ps[:],
                            lhsT=aT6[:, 2 * dp:2 * dp + 2, :],
                            rhs=wo8[dp][:, :, half * NB:(half + 1) * NB],
                            start=(dp == 0),
                            stop=(dp == DTP - 1),
                            perf_mode=DR,
                        )
                    nc.vector.tensor_add(
                        y_sb[:, half * NB:(half + 1) * NB],
                        ps[:],
                        x32[:, half * NB:(half + 1) * NB],
                    )
                # LayerNorm over the 1024 free dim (bout pre-folded into x_own)
                st = stE.tile([P, 2, 6], F32, tag="bn")
                nc.vector.bn_stats(st[:, 0, :], y_sb[:, 0:NB])
                nc.vector.bn_stats(st[:, 1, :], y_sb[:, NB:DIM])
                mv = stE.tile([P, 2], F32, tag="mv")
                nc.vector.bn_aggr(mv[:], st[:])
                nm = stE.tile([P, 1], F32, tag="nm")
                nc.vector.tensor_scalar_mul(nm[:], mv[:, 0:1], -1.0)
                rstd = stE.tile([P, 1], F32, tag="rstd")
                nc.scalar.activation(rstd[:], mv[:, 1:2], SQRT, bias=eps_t[:],
                                     scale=1.0)
                nc.vector.reciprocal(rstd[:], rstd[:])
                nc.vector.tensor_scalar(
                    out=y_sb[:], in0=y_sb[:], scalar1=nm[:], scalar2=rstd[:],
                    op0=ADD, op1=MULT,
                )
                nc.vector.tensor_mul(y_sb[:], y_sb[:], gamma_rep[:])
                nc.vector.tensor_add(y_sb[:], y_sb[:], beta_rep[:])
                nc.sync.dma_start(out=y_ext[tsl, :], in_=y_sb[:])

    _split_sync_waits(nc)
    return nc


_NC_CACHE = None
LAST_RESULT = None


def _get_nc():
    global _NC_CACHE
    if _NC_CACHE is None:
        _NC_CACHE = _build_bass()
    return _NC_CACHE


def _to_f8(a):
    return np.ascontiguousarray(
        np.clip(np.asarray(a, dtype=np.float32), -240.0, 240.0).astype(NPF8)
    )


def kernel(embedding1, embedding2, Wqkv, bqkv, Wout, bout, gamma, beta):
    global LAST_RESULT
    embs = [np.ascontiguousarray(np.asarray(embedding1, dtype=np.float32)),
            np.ascontiguousarray(np.asarray(embedding2, dtype=np.float32))]
    w = np.asarray(Wqkv, dtype=np.float32)
    w8 = np.concatenate(
        [SC_QK * w[:, 0:DIM], SC_QK * w[:, DIM:2 * DIM], SC_V * w[:, 2 * DIM:]],
        axis=1,
    )
    w8 = _to_f8(w8)
    wo8 = _to_f8(SC_WO * np.asarray(Wout, dtype=np.float32))
    bq = np.asarray(bqkv, dtype=np.float32).reshape(3 * DIM).copy()
    bq[0:2 * DIM] *= SC_QK
    bq[2 * DIM:] *= SC_V
    bq = np.ascontiguousarray(bq)
    bo = np.ascontiguousarray(
        SC_X * np.asarray(bout, dtype=np.float32)).reshape(1, DIM)
    ga = np.ascontiguousarray(
        np.asarray(gamma, dtype=np.float32).astype(np.float16)).reshape(1, DIM)
    be = np.ascontiguousarray(
        np.asarray(beta, dtype=np.float32).astype(np.float16)).reshape(1, DIM)
    # host-side layout prep: fp8 cast + transpose (dim-major) per (stream, batch)
    xT = [[_to_f8(embs[s][b].T) for b in range(2)] for s in range(2)]

    nc = _get_nc()
    in_maps = []
    layout = []  # (s, b, c) per core
    for s in range(2):
        for b in range(2):
            for c in range(2):
                in_maps.append({
                    "x_own": np.ascontiguousarray(
                        (SC_X * (embs[s][b, c * NQ:(c + 1) * NQ, :]
                                 + np.asarray(bout, dtype=np.float32)
                                 .reshape(1, DIM))).astype(np.float16)),
                    "xT8": np.ascontiguousarray(xT[s][b][:, c * NQ:(c + 1) * NQ]),
                    "xoT8": xT[1 - s][b],
                    "wqkv8": w8,
                    "wout8i": wo8,
                    "bqkv": bq,
                    "bout": bo,
                    "gamma": ga,
                    "beta": be,
                })
                layout.append((s, b, c))

    trace = os.environ.get("TRN_KERNEL_TRACE", "") not in ("", "0")
    if trace:
        _ensure_ntff_hook()
        _patch_upload_artifacts()
    res = run_bass_kernel_spmd(
        nc, in_maps, core_ids=list(range(8)), trace=trace,
    )
    LAST_RESULT = res

    out = np.zeros((2, 2, N_TOK, DIM), dtype=np.float32)
    for i, (s, b, c) in enumerate(layout):
        out[s, b, c * NQ:(c + 1) * NQ, :] = np.asarray(
            res.results[i]["y"]).astype(np.float32)
    return out


# revision 23
# speedup vs baseline: 1.1749x; 1.1749x over previous
"""Trainium2 Bass kernel for nn_CrossAttentionLayer (2-stream cross-attention + LN).

Sharding: 8 cores = (stream s in {0,1}) x (batch b in {0,1}) x (query chunk c in {0,1}).
Each core handles 1024 query tokens of one (stream, batch): it projects Q for its
tokens, K/V for the *other* stream's full 2048 tokens (data-parallel duplication of
KV-proj across the 2 chunk cores), runs 16-head cross attention, out-projection,
residual and LayerNorm, and returns its [1024, 1024] slice.

Perf scheme (v2):
- All projections (Q/K/V/out) and the AV matmul run in fp8e4 with DoubleRow
  perf mode (2 contraction subtiles per instruction, 0.5 cycles/row = 2x fp16
  matmul throughput). Scores stay at 1.0 cycles/row (contraction=64 head dim),
  operands fp8.
- Weights are pre-scaled on the host so fp8e4 (max 240, min normal 2^-6) sees
  ~unit-scale values: Wq,Wk x64, Wv,Wout x32. Scale bookkeeping:
  Q' K' = 4096 QK -> folded into the exp() scale argument; V' = 32V and
  Wout' = 32 Wout -> attn-out psum = 1024 attn@Wout, matched by x_own and bout
  pre-scaled x1024 on the host. The final LayerNorm is scale-invariant, so the
  output is exact.
- exp() on the ACT engine writes fp8 directly (the ACT engine is the critical
  path: 33.5M softmax exps/core ~ 220us minimum).
- Softmax normalization: reciprocal_approx_fast (5x faster than
  nc.vector.reciprocal) + gpsimd partition_broadcast (Pool engine) instead of
  a DRAM bounce.
- Single interleaved region: K(f)/Q(f) projections feed scores for head pair
  g=f immediately, so the ACT engine starts exp'ing ~10us in and stays busy;
  V projection and AV/out-proj ride in PE slack under the exp stream.
"""

import os
import sys

import numpy as np

for _p in ("/opt/trn_rl_repo", "/root/.axon_site/_ro/trn_rl_repo"):
    if os.path.isdir(_p) and _p not in sys.path:
        sys.path.insert(0, _p)

import ml_dtypes

import concourse.bass as bass
import concourse.mybir as mybir
import concourse.tile as tile
from concourse.bass_utils import run_bass_kernel_spmd

F32 = mybir.dt.float32
F16 = mybir.dt.float16
F8 = mybir.dt.float8e4
NPF8 = ml_dtypes.float8_e4m3
ADD = mybir.AluOpType.add
MULT = mybir.AluOpType.mult
EXP = mybir.ActivationFunctionType.Exp
SQRT = mybir.ActivationFunctionType.Sqrt
DR = mybir.MatmulPerfMode.DoubleRow

DIM = 1024
N_TOK = 2048
HEADS = 16
HD = DIM // HEADS        # 64
NQ = 1024                # query tokens per core
S = 2048                 # kv sequence length
P = 128
DT = DIM // P            # 8 contraction tiles
DTP = DT // 2            # 4 contraction tile-pairs (DoubleRow)
FT = DIM // P            # 8 feature tiles
KT = S // P              # 16 key tiles
KT2 = KT // 2            # 8 key tile-pairs
NB = 512                 # matmul free-dim / psum bank width (fp32)
QC = NQ // NB            # 2 query chunks
KC = S // NB             # 4 key chunks
TT = NQ // P             # 8 token tiles per core
SCALE = HD ** -0.5
EPS = 1e-5

# host-side fp8 pre-scales (powers of two; see module docstring)
SC_QK = 64.0             # Wq, Wk, bq, bk
SC_V = 32.0              # Wv, bv
SC_WO = 32.0             # Wout
SC_X = SC_V * SC_WO      # x_own, bout (1024)
EXP_SCALE = SCALE / (SC_QK * SC_QK)

_wsplit_ctr = [0]


def _ensure_ntff_hook():
    """Register the axon NTFF profiling hook if the image lacks
    antenv.axon_hooks (mirrors trn_boot._ntff_profile_via_ctypes)."""
    try:
        from antenv.axon_hooks import get_axon_ntff_profile_hook  # noqa: F401
        return
    except ImportError:
        pass
    import contextlib
    import ctypes
    import types

    try:
        import antenv
    except ImportError:
        return
    mod = types.ModuleType("antenv.axon_hooks")
    _h = [None]
    mod.set_axon_ntff_profile_hook = lambda h: _h.__setitem__(0, h)
    mod.get_axon_ntff_profile_hook = lambda: _h[0]
    sys.modules["antenv.axon_hooks"] = mod
    antenv.axon_hooks = mod

    so_path = "/opt/axon/libaxon_pjrt.so"
    if not os.path.exists(so_path):
        return
    try:
        lib = ctypes.CDLL(so_path)
    except OSError:
        return
    if not hasattr(lib, "axon_start_nrt_profile"):
        return
    lib.axon_start_nrt_profile.argtypes = [
        ctypes.POINTER(ctypes.c_int64),
        ctypes.c_size_t,
    ]
    lib.axon_start_nrt_profile.restype = ctypes.c_int64
    lib.axon_stop_nrt_profile.argtypes = [ctypes.c_char_p]
    lib.axon_stop_nrt_profile.restype = ctypes.c_int64

    @contextlib.contextmanager
    def _hook(output_dir, device_ids):
        import jax

        jax.devices()
        if device_ids:
            ids = (ctypes.c_int64 * len(device_ids))(*device_ids)
            rc = lib.axon_start_nrt_profile(ids, len(device_ids))
        else:
            rc = lib.axon_start_nrt_profile(None, 0)
        if rc != 0:
            raise RuntimeError(f"axon_start_nrt_profile rc={rc}")
        try:
            yield
        finally:
            n = lib.axon_stop_nrt_profile(str(output_dir).encode())
            if n <= 0:
                print(f"profile: rc={n}, no ntff written to {output_dir}")

    mod.set_axon_ntff_profile_hook(_hook)


def _patch_upload_artifacts():
    """Artifact upload needs bucket access this container may not have;
    neuter it (only reachable on trace paths)."""
    from concourse import bass_utils as bu

    bu.upload_artifacts = lambda tmpdir: str(tmpdir)


def _split_sync_waits(nc):
    """This container's walrus build rejects >1 sync-wait per instruction.
    Hoist extra waits onto same-engine NOPs placed just before the instruction
    (engines execute their stream in order, so semantics are preserved)."""
    for f in nc.m.functions:
        for bb in f.blocks:
            insts = bb.instructions
            out = []
            changed = False
            for inst in insts:
                si = inst.sync_info
                if si is not None and si.on_wait and len(si.on_wait) > 1:
                    waits = list(si.on_wait)
                    for w in waits[:-1]:
                        _wsplit_ctr[0] += 1
                        out.append(
                            mybir.InstNoOp(
                                name=f"I-wsplit-{_wsplit_ctr[0]}",
                                engine=inst.engine,
                                ins=[],
                                outs=[],
                                sync_info=mybir.SyncInfo(on_wait=[w], on_update=[]),
                            )
                        )
                    si.on_wait = waits[-1:]
                    changed = True
                out.append(inst)
            if changed:
                insts[:] = out
    return nc


def _build_bass():
    nc = bass.Bass()
    x_own = nc.declare_dram_parameter("x_own", [NQ, DIM], F16, isOutput=False)
    xT8 = nc.declare_dram_parameter("xT8", [DIM, NQ], F8, isOutput=False)
    xoT8 = nc.declare_dram_parameter("xoT8", [DIM, S], F8, isOutput=False)
    wqkv8 = nc.declare_dram_parameter("wqkv8", [DIM, 3 * DIM], F8, isOutput=False)
    wout8i = nc.declare_dram_parameter("wout8i", [DIM, DIM], F8, isOutput=False)
    bqkv = nc.declare_dram_parameter("bqkv", [3 * DIM], F32, isOutput=False)
    bout = nc.declare_dram_parameter("bout", [1, DIM], F32, isOutput=False)
    gamma = nc.declare_dram_parameter("gamma", [1, DIM], F16, isOutput=False)
    beta = nc.declare_dram_parameter("beta", [1, DIM], F16, isOutput=False)
    y_ext = nc.declare_dram_parameter("y", [NQ, DIM], F16, isOutput=True)

    with tile.TileContext(nc, pool_alloc_mode="queue") as tc:
        from contextlib import ExitStack

        with ExitStack() as ctx:
            const = ctx.enter_context(tc.tile_pool(name="const", bufs=1))
            persist = ctx.enter_context(tc.tile_pool(name="persist", bufs=1))
            dram = ctx.enter_context(tc.tile_pool(name="dram", bufs=1, space="DRAM"))
            dramn = ctx.enter_context(tc.tile_pool(name="dramn", bufs=3, space="DRAM"))
            # PSUM during attention: psP 2 + psS 2*2 + psAt 2 = 8 banks
            psP = ctx.enter_context(tc.tile_pool(name="psP", bufs=2, space="PSUM"))
            pTp = ctx.enter_context(tc.tile_pool(name="pT", bufs=32))
            rrp = ctx.enter_context(tc.tile_pool(name="rr", bufs=4))
            rdp = ctx.enter_context(tc.tile_pool(name="rd", bufs=4))
            asg = ctx.enter_context(tc.tile_pool(name="asg", bufs=5))
            stE = ctx.enter_context(tc.tile_pool(name="stE", bufs=3))
            xrp = ctx.enter_context(tc.tile_pool(name="xrp", bufs=8))
            aTE = ctx.enter_context(tc.tile_pool(name="aTE", bufs=8))

            # ---- constants (broadcast along partitions via DMA) ----
            bq_cols = const.tile([P, 3 * DT], F32)  # bqkv as feat-major columns
            nc.gpsimd.dma_start(out=bq_cols[:], in_=bqkv[:].rearrange("(t p) -> p t", p=P))
            bv_rep = const.tile([P, DIM], F32)
            nc.gpsimd.dma_start(
                out=bv_rep[:],
                in_=bass.AP(tensor=bqkv[:].tensor, offset=2 * DIM, ap=[[0, P], [1, DIM]]),
            )
            gamma_rep = const.tile([P, DIM], F16)
            nc.gpsimd.dma_start(out=gamma_rep[:], in_=gamma[:].to_broadcast([P, DIM]))
            beta_rep = const.tile([P, DIM], F16)
            nc.gpsimd.dma_start(out=beta_rep[:], in_=beta[:].to_broadcast([P, DIM]))
            eps_t = const.tile([P, 1], F32)
            nc.vector.memset(eps_t[:], EPS)

            # ---- persistent operands ----
            # K/Q stored fp8, feature-major [feat 128, tokens]
            kTs = [persist.tile([P, S], F8, name=f"kT{f}") for f in range(FT)]
            qTs = [persist.tile([P, NQ], F8, name=f"qT{f}") for f in range(FT)]
            # V in fp8, DoubleRow pair layout: [key 128, ktile j, head, hd+1]
            vSs = [persist.tile([P, 2, HEADS, HD + 1], F8, name=f"vS{k2}")
                   for k2 in range(KT2)]
            # fp8 inputs (DoubleRow pair layout [128, 2, n])
            wk8a = persist.tile([P, DT, DIM], F8, name="wk8a")
            wq8a = persist.tile([P, DT, DIM], F8, name="wq8a")
            wv8a = persist.tile([P, DT, DIM], F8, name="wv8a")
            wo8a = persist.tile([P, DT, DIM], F8, name="wo8a")
            xo8a = persist.tile([P, DT, S], F8, name="xo8a")
            x8a = persist.tile([P, DT, NQ], F8, name="x8a")
            wk8 = [wk8a[:, 2 * dp:2 * dp + 2, :] for dp in range(DTP)]
            wq8 = [wq8a[:, 2 * dp:2 * dp + 2, :] for dp in range(DTP)]
            wv8 = [wv8a[:, 2 * dp:2 * dp + 2, :] for dp in range(DTP)]
            wo8 = [wo8a[:, 2 * dp:2 * dp + 2, :] for dp in range(DTP)]
            xo8 = [xo8a[:, 2 * dp:2 * dp + 2, :] for dp in range(DTP)]
            x8 = [x8a[:, 2 * dp:2 * dp + 2, :] for dp in range(DTP)]

            # ---- input DMA loads: one DMA per tensor, spread across issue
            # queues; K-path (sync) first so scores start earliest ----
            nc.sync.dma_start(
                out=xo8a[:], in_=xoT8[:, :].rearrange("(r p) n -> p r n", p=P))
            nc.scalar.dma_start(
                out=wk8a[:],
                in_=wqkv8[:, DIM:2 * DIM].rearrange("(r p) n -> p r n", p=P))
            nc.scalar.dma_start(
                out=wq8a[:],
                in_=wqkv8[:, 0:DIM].rearrange("(r p) n -> p r n", p=P))
            nc.scalar.dma_start(
                out=x8a[:], in_=xT8[:, :].rearrange("(r p) n -> p r n", p=P))
            nc.gpsimd.dma_start(
                out=wv8a[:],
                in_=wqkv8[:, 2 * DIM:3 * DIM].rearrange("(r p) n -> p r n", p=P))
            nc.gpsimd.dma_start(
                out=wo8a[:], in_=wout8i[:, :].rearrange("(r p) n -> p r n", p=P))
            # residual rows: independent of everything, preload now
            xrp_t = []
            for t in range(TT):
                xr = xrp.tile([P, DIM], F16, tag="xr", name=f"xr{t}")
                nc.gpsimd.dma_start(out=xr[:], in_=x_own[t * P:(t + 1) * P, :])
                xrp_t.append(xr)

            attn_d = dram.tile([DIM, NQ], F8, name="attnd")

            # ones rows for the softmax denominator (independent bytes of vSs)
            for k2 in range(KT2):
                nc.vector.memset(vSs[k2][:, :, :, HD:HD + 1], 1.0)

            # ---- filler units: single-shot closures emitting ~4 PE matmuls
            # each; woven between exp-paced score rounds to keep the PE dense.
            from collections import deque
            fillers = deque()

            def drain(n):
                for _ in range(n):
                    if fillers:
                        fillers.popleft()()

            def proj_k_unit(f, kc):
                def emit():
                    ps = psP.tile([P, NB], F32, tag="ps", name="ps")
                    for dp in range(DTP):
                        nc.tensor.matmul(
                            ps[:],
                            lhsT=wk8[dp][:, :, f * P:(f + 1) * P],
                            rhs=xo8[dp][:, :, kc * NB:(kc + 1) * NB],
                            start=(dp == 0),
                            stop=(dp == DTP - 1),
                            perf_mode=DR,
                        )
                    nc.vector.tensor_scalar(
                        out=kTs[f][:, kc * NB:(kc + 1) * NB],
                        in0=ps[:],
                        scalar1=bq_cols[:, DT + f:DT + f + 1],
                        scalar2=None,
                        op0=ADD,
                    )
                return emit

            def proj_q_unit(f, q):
                def emit():
                    ps = psP.tile([P, NB], F32, tag="ps", name="ps")
                    for dp in range(DTP):
                        nc.tensor.matmul(
                            ps[:],
                            lhsT=wq8[dp][:, :, f * P:(f + 1) * P],
                            rhs=x8[dp][:, :, q * NB:(q + 1) * NB],
                            start=(dp == 0),
                            stop=(dp == DTP - 1),
                            perf_mode=DR,
                        )
                    nc.vector.tensor_scalar(
                        out=qTs[f][:, q * NB:(q + 1) * NB],
                        in0=ps[:],
                        scalar1=bq_cols[:, f:f + 1],
                        scalar2=None,
                        op0=ADD,
                    )
                return emit

            def proj_v_unit(k2, j, half):
                kt = 2 * k2 + j

                def emit():
                    ps = psP.tile([P, NB], F32, tag="ps", name="ps")
                    for dp in range(DTP):
                        nc.tensor.matmul(
                            ps[:],
                            lhsT=xo8[dp][:, :, kt * P:(kt + 1) * P],
                            rhs=wv8[dp][:, :, half * NB:(half + 1) * NB],
                            start=(dp == 0),
                            stop=(dp == DTP - 1),
                            perf_mode=DR,
                        )
                    nc.vector.tensor_add(
                        vSs[k2][:, j, half * 8:(half + 1) * 8, 0:HD],
                        ps[:].rearrange("p (h d) -> p h d", d=HD),
                        bv_rep[:, half * NB:(half + 1) * NB].rearrange(
                            "p (h d) -> p h d", d=HD
                        ),
                    )
                return emit

            def av_unit(g, q, hi, pts, sink):
                def emit():
                    ps_at = psAt.tile([HD + 1, NB], F32, tag="psa",
                                      name=f"psa{g}_{q}_{hi}")
                    for k2 in range(KT2):
                        nc.tensor.matmul(
                            ps_at[:],
                            lhsT=vSs[k2][:, :, 2 * g + hi, :],
                            rhs=pts[k2][hi][:],
                            start=(k2 == 0),
                            stop=(k2 == KT2 - 1),
                            perf_mode=DR,
                        )
                    a32 = asg.tile([HD + 1, NB], F32, tag="a32", name="a32")
                    nc.vector.tensor_copy(a32[:], ps_at[:])
                    sink["a32"][(q, hi)] = a32
                    nc.gpsimd.dma_start(
                        out=sink["den_d"][2 * q + hi:2 * q + hi + 1, :],
                        in_=a32[HD:HD + 1, :],
                    )
                return emit

            def norm_unit(g, sink):
                """Batched reciprocal of this head pair's 4 denominators via a
                DRAM bounce reshaped to [128, 16], then normalize + store."""
                def emit():
                    rd4 = rdp.tile([P, 16], F32, tag="rd4", name="rd4")
                    dflat = sink["den_d"][:]
                    nc.gpsimd.dma_start(
                        out=rd4[:],
                        in_=bass.AP(tensor=dflat.tensor, offset=dflat.offset,
                                    ap=[[16, P], [1, 16]]),
                    )
                    nc.vector.reciprocal(rd4[:], rd4[:])
                    rec_d = dramn.tile([4, NB], F32, tag="recd", name="recd")
                    rflat = rec_d[:]
                    nc.gpsimd.dma_start(
                        out=bass.AP(tensor=rflat.tensor, offset=rflat.offset,
                                    ap=[[16, P], [1, 16]]),
                        in_=rd4[:],
                    )
                    for q in range(QC):
                        qsl = slice(q * NB, (q + 1) * NB)
                        for hi in range(2):
                            h = 2 * g + hi
                            rrep = rrp.tile([HD, NB], F32, tag="rrep", name="rrep")
                            nc.gpsimd.dma_start(
                                out=rrep[:],
                                in_=rec_d[2 * q + hi:2 * q + hi + 1, :]
                                .to_broadcast([HD, NB]),
                            )
                            a8 = asg.tile([HD, NB], F8, tag="a8", name="a8")
                            nc.vector.tensor_mul(
                                a8[:], sink["a32"][(q, hi)][0:HD, :], rrep[:]
                            )
                            nc.gpsimd.dma_start(
                                out=attn_d[h * HD:(h + 1) * HD, qsl], in_=a8[:]
                            )
                return emit

            def queue_av(g, pts_q):
                den_d = dramn.tile([4, NB], F32, tag="dend", name="dend")
                sink = {"a32": {}, "den_d": den_d}
                for q in range(QC):
                    for hi in range(2):
                        fillers.append(av_unit(g, q, hi, pts_q[q], sink))
                return sink

            with tc.tile_pool(name="psS", bufs=2, space="PSUM") as psS, \
                 tc.tile_pool(name="psAt", bufs=2, space="PSUM") as psAt:

                def scores(g, q, pts, per_round=1):
                    """QK^T + exp for head pair g, query chunk q; filler
                    units woven between kt2 rounds."""
                    f = g
                    qsl = slice(q * NB, (q + 1) * NB)
                    sched = (per_round if isinstance(per_round, list)
                             else [per_round] * KT2)
                    for k2 in range(KT2):
                        ps_s = [
                            psS.tile([P, 2, NB], F32, tag="pss",
                                     name=f"pss{g}_{q}_{k2}_{i}")
                            for i in range(2)
                        ]
                        for j in range(2):
                            kt = 2 * k2 + j
                            for hi in range(2):
                                po = hi * HD
                                nc.tensor.matmul(
                                    ps_s[hi][:, j, :],
                                    lhsT=kTs[f][po:po + HD, kt * P:(kt + 1) * P],
                                    rhs=qTs[f][po:po + HD, qsl],
                                    start=True,
                                    stop=True,
                                )
                        drain(sched[k2])
                        pp = []
                        for hi in range(2):
                            pt = pTp.tile([P, 2, NB], F8, tag="pT", name="pt")
                            nc.scalar.activation(pt[:], ps_s[hi][:], EXP,
                                                 scale=EXP_SCALE)
                            pp.append(pt)
                        pts.append(pp)

                # ---- weave driver ----
                # Ordering invariants (PE queue is in-order, so a stalled
                # instruction blocks everything behind it):
                #  - K(g)/Q(g) units fully emitted before scores(g) starts.
                #  - AV(g-1) units emitted in the FIRST rounds of scores(g):
                #    exp(g) tiles WAR-wait on the pt ring buffers that
                #    AV(g-1) reads (ring size = one head pair's tiles).
                #  - All V units emitted before AV(0) (drain 2/round in S(0)).
                pts_all = {}
                for kc in range(KC):
                    proj_k_unit(0, kc)()
                for q in range(QC):
                    proj_q_unit(0, q)()
                for k2 in range(KT2):
                    for j in range(2):
                        for half in range(2):
                            fillers.append(proj_v_unit(k2, j, half))
                fillers.append(proj_k_unit(1, 0))
                fillers.append(proj_k_unit(1, 1))
                fillers.append(proj_k_unit(1, 2))
                fillers.append(proj_k_unit(1, 3))
                fillers.append(proj_q_unit(1, 0))
                fillers.append(proj_q_unit(1, 1))
                pts_all[(0, 0)] = []
                scores(0, 0, pts_all[(0, 0)], per_round=[0, 0, 1, 2, 3, 3, 3, 3])
                pts_all[(0, 1)] = []
                scores(0, 1, pts_all[(0, 1)], per_round=3)
                for g in range(1, HEADS // 2):
                    drain(len(fillers))  # flush: K(g)/Q(g) done before S(g)
                    sink = queue_av(g - 1, [pts_all[(g - 1, q)] for q in range(QC)])
                    if g + 1 < HEADS // 2:
                        for kc in range(KC):
                            fillers.append(proj_k_unit(g + 1, kc))
                        for q in range(QC):
                            fillers.append(proj_q_unit(g + 1, q))
                    fillers.append(norm_unit(g - 1, sink))
                    for q in range(QC):
                        pts_all[(g, q)] = []
                        scores(g, q, pts_all[(g, q)])
                    for q in range(QC):
                        pts_all.pop((g - 1, q))
                sink = queue_av(HEADS // 2 - 1,
                                [pts_all[(HEADS // 2 - 1, q)] for q in range(QC)])
                fillers.append(norm_unit(HEADS // 2 - 1, sink))
                drain(len(fillers))

            # ======== out proj + residual + LN ========
            psE = ctx.enter_context(tc.tile_pool(name="psE", bufs=3, space="PSUM"))
            aT6s = []
            for t in range(TT):
                aT6 = aTE.tile([P, FT, P], F8, tag="aT", name=f"aT6_{t}")
                nc.sync.dma_start(
                    out=aT6[:],
                    in_=attn_d[:].rearrange("(f p) t -> p f t", p=P)
                    [:, :, t * P:(t + 1) * P],
                )
                aT6s.append(aT6)
            for t in range(TT):
                tsl = slice(t * P, (t + 1) * P)
                x32 = xrp_t[t]
                y_sb = stE.tile([P, DIM], F16, tag="ysb")
                aT6 = aT6s[t]
                for half in range(2):
                    ps = psE.tile([P, NB], F32, tag="ps")
                    for dp in range(DTP):
                        nc.tensor.matmul(
                            ps[:],
                            lhsT=aT6[:, 2 * dp:2 * dp + 2, :],
                            rhs=wo8[dp][:, :, half * NB:(half + 1) * NB],
                            start=(dp == 0),
                            stop=(dp == DTP - 1),
                            perf_mode=DR,
                        )
                    nc.vector.tensor_add(
                        y_sb[:, half * NB:(half + 1) * NB],
                        ps[:],
                        x32[:, half * NB:(half + 1) * NB],
                    )
                # LayerNorm over the 1024 free dim (bout pre-folded into x_own)
                st = stE.tile([P, 2, 6], F32, tag="bn")
                nc.vector.bn_stats(st[:, 0, :], y_sb[:, 0:NB])
                nc.vector.bn_stats(st[:, 1, :], y_sb[:, NB:DIM])
                mv = stE.tile([P, 2], F32, tag="mv")
                nc.vector.bn_aggr(mv[:], st[:])
                nm = stE.tile([P, 1], F32, tag="nm")
                nc.vector.tensor_scalar_mul(nm[:], mv[:, 0:1], -1.0)
                rstd = stE.tile([P, 1], F32, tag="rstd")
                nc.scalar.activation(rstd[:], mv[:, 1:2], SQRT, bias=eps_t[:],
                                     scale=1.0)
                nc.vector.reciprocal(rstd[:], rstd[:])
                nc.vector.tensor_scalar(
                    out=y_sb[:], in0=y_sb[:], scalar1=nm[:], scalar2=rstd[:],
                    op0=ADD, op1=MULT,
                )
                nc.vector.tensor_mul(y_sb[:], y_sb[:], gamma_rep[:])
                nc.vector.tensor_add(y_sb[:], y_sb[:], beta_rep[:])
                nc.sync.dma_start(out=y_ext[tsl, :], in_=y_sb[:])

    _split_sync_waits(nc)
    return nc


_NC_CACHE = None
LAST_RESULT = None


def _get_nc():
    global _NC_CACHE
    if _NC_CACHE is None:
        _NC_CACHE = _build_bass()
    return _NC_CACHE


def _to_f8(a):
    return np.ascontiguousarray(
        np.clip(np.asarray(a, dtype=np.float32), -240.0, 240.0).astype(NPF8)
    )


def kernel(embedding1, embedding2, Wqkv, bqkv, Wout, bout, gamma, beta):
    global LAST_RESULT
    embs = [np.ascontiguousarray(np.asarray(embedding1, dtype=np.float32)),
            np.ascontiguousarray(np.asarray(embedding2, dtype=np.float32))]
    w = np.asarray(Wqkv, dtype=np.float32)
    w8 = np.concatenate(
        [SC_QK * w[:, 0:DIM], SC_QK * w[:, DIM:2 * DIM], SC_V * w[:, 2 * DIM:]],
        axis=1,
    )
    w8 = _to_f8(w8)
    wo8 = _to_f8(SC_WO * np.asarray(Wout, dtype=np.float32))
    bq = np.asarray(bqkv, dtype=np.float32).reshape(3 * DIM).copy()
    bq[0:2 * DIM] *= SC_QK
    bq[2 * DIM:] *= SC_V
    bq = np.ascontiguousarray(bq)
    bo = np.ascontiguousarray(
        SC_X * np.asarray(bout, dtype=np.float32)).reshape(1, DIM)
    ga = np.ascontiguousarray(
        np.asarray(gamma, dtype=np.float32).astype(np.float16)).reshape(1, DIM)
    be = np.ascontiguousarray(
        np.asarray(beta, dtype=np.float32).astype(np.float16)).reshape(1, DIM)
    # host-side layout prep: fp8 cast + transpose (dim-major) per (stream, batch)
    xT = [[_to_f8(embs[s][b].T) for b in range(2)] for s in range(2)]

    nc = _get_nc()
    in_maps = []
    layout = []  # (s, b, c) per core
    for s in range(2):
        for b in range(2):
            for c in range(2):
                in_maps.append({
                    "x_own": np.ascontiguousarray(
                        (SC_X * (embs[s][b, c * NQ:(c + 1) * NQ, :]
                                 + np.asarray(bout, dtype=np.float32)
                                 .reshape(1, DIM))).astype(np.float16)),
                    "xT8": np.ascontiguousarray(xT[s][b][:, c * NQ:(c + 1) * NQ]),
                    "xoT8": xT[1 - s][b],
                    "wqkv8": w8,
                    "wout8i": wo8,
                    "bqkv": bq,
                    "bout": bo,
                    "gamma": ga,
                    "beta": be,
                })
                layout.append((s, b, c))

    trace = os.environ.get("TRN_KERNEL_TRACE", "") not in ("", "0")
    if trace:
        _ensure_ntff_hook()
        _patch_upload_artifacts()
    res = run_bass_kernel_spmd(
        nc, in_maps, core_ids=list(range(8)), trace=trace,
    )
    LAST_RESULT = res

    out = np.zeros((2, 2, N_TOK, DIM), dtype=np.float32)
    for i, (s, b, c) in enumerate(layout):
        out[s, b, c * NQ:(c + 1) * NQ, :] = np.asarray(
            res.results[i]["y"]).astype(np.float32)
    return out
